# Initial kernel scaffold
#
import sys

sys.path.insert(0, "/opt/trn_rl_repo")

import math

import numpy as np

import concourse.bass as bass
import concourse.mybir as mybir
import concourse.tile as tile
from concourse import bacc
from concourse.bass_utils import run_bass_kernel_spmd
from concourse.masks import make_identity

F32 = mybir.dt.float32
F32R = mybir.dt.float32r
AF = mybir.ActivationFunctionType
OP = mybir.AluOpType

N_CORES = 8
B_TOTAL = 131072
B = B_TOTAL // N_CORES  # 16384 rows per core
P = 128
NCH = B // P  # 128 chunks of 128 rows
LAT = 16
CIN = 17
HID = 256
NJ = 7

SC = 2048          # super-chunk width (b' columns)
NSC = B // SC      # 8 super-chunks
CH_PER_SC = SC // P  # 16 x-chunks per super-chunk
NN_PER_SC = SC // 512  # 4 N-chunks of 512

PI = math.pi

# Franka DH constants
DH_A = [0.0, 0.0, 0.0, 0.0825, -0.0825, 0.0, 0.088]
DH_D = [0.333, 0.0, 0.316, 0.0, 0.384, 0.0, 0.0]
DH_SA = [0, -1, 1, 1, -1, 1, 1]  # sin(alpha), exact

USE_F32R = True


def _r(ap):
    """View an f32 AP as float32r for full-rate PE matmuls."""
    if USE_F32R:
        return ap.bitcast(F32R)
    return ap


# ----------------------------------------------------------------------------
# FK symbolic builder: entries are Zero, Const, or Tile(ap, sigma)
# ----------------------------------------------------------------------------
class E:
    __slots__ = ("kind", "val", "ap", "sg")

    def __init__(self, kind, val=0.0, ap=None, sg=1):
        self.kind = kind  # 'Z' | 'C' | 'T'
        self.val = val
        self.ap = ap
        self.sg = sg


ZERO = E("Z")


def C(v):
    return E("C", val=v)


def T(ap, sg=1):
    return E("T", ap=ap, sg=sg)


class FKB:
    """Emits bass ops for the FK chain with compile-time constant folding."""

    def __init__(self, nc, pool, nb):
        self.nc = nc
        self.pool = pool
        self.nb = nb
        self.n_tt = 0  # op counters
        self.n_ts = 0
        self.rr = 0

    def fresh(self, out=None):
        if out is not None:
            return out
        return self.pool.tile([P, self.nb], F32, tag="fk")[:]

    def _veng(self):
        # round-robin heavy 2-input ops between DVE and GPSIMD
        self.rr += 1
        return self.nc.vector if (self.rr % 3) else self.nc.gpsimd

    def mul_trig(self, x, trig_ap):
        """entry * trig tile -> entry"""
        nc = self.nc
        if x.kind == "Z":
            return ZERO
        if x.kind == "C":
            o = self.fresh()
            nc.gpsimd.tensor_scalar_mul(o, trig_ap, float(x.val))
            self.n_ts += 1
            return T(o)
        o = self.fresh()
        self._veng().tensor_tensor(o, x.ap, trig_ap, OP.mult)
        self.n_tt += 1
        return T(o, x.sg)

    def lincomb(self, a, b, out=None):
        """a + b (entries with signs) -> entry (one TT op when both tiles)."""
        nc = self.nc
        if a.kind == "Z":
            if out is not None and b.kind == "T":
                nc.gpsimd.tensor_scalar_mul(out, b.ap, float(b.sg))
                self.n_ts += 1
                return T(out)
            return b
        if b.kind == "Z":
            if out is not None and a.kind == "T":
                nc.gpsimd.tensor_scalar_mul(out, a.ap, float(a.sg))
                self.n_ts += 1
                return T(out)
            return a
        assert a.kind == "T" and b.kind == "T"
        o = self.fresh(out)
        eng = self._veng()
        if a.sg > 0 and b.sg > 0:
            eng.tensor_tensor(o, a.ap, b.ap, OP.add)
            sg = 1
        elif a.sg > 0 and b.sg < 0:
            eng.tensor_tensor(o, a.ap, b.ap, OP.subtract)
            sg = 1
        elif a.sg < 0 and b.sg > 0:
            eng.tensor_tensor(o, b.ap, a.ap, OP.subtract)
            sg = 1
        else:
            eng.tensor_tensor(o, a.ap, b.ap, OP.add)
            sg = -1
        self.n_tt += 1
        return T(o, sg)

    def neg(self, a):
        if a.kind == "Z":
            return ZERO
        if a.kind == "C":
            return C(-a.val)
        return T(a.ap, -a.sg)

    def axpw(self, a_const, x, w, out=None):
        """a_const * x + w -> entry with sigma=+ (true value)."""
        nc = self.nc
        if x.kind == "Z" or a_const == 0.0:
            # result = w
            if w.kind == "T" and out is not None:
                nc.gpsimd.tensor_scalar_mul(out, w.ap, float(w.sg))
                self.n_ts += 1
                return T(out)
            return w
        if x.kind == "C":
            cv = a_const * x.val
            if w.kind == "Z":
                return C(cv)
            if w.kind == "C":
                return C(cv + w.val)
            o = self.fresh(out)
            nc.gpsimd.tensor_scalar(o, w.ap, float(w.sg), cv, OP.mult, OP.add)
            self.n_ts += 1
            return T(o)
        a_eff = a_const * x.sg
        if w.kind == "Z":
            o = self.fresh(out)
            nc.gpsimd.tensor_scalar_mul(o, x.ap, float(a_eff))
            self.n_ts += 1
            return T(o)
        if w.kind == "C":
            o = self.fresh(out)
            nc.gpsimd.tensor_scalar(o, x.ap, float(a_eff), float(w.val), OP.mult, OP.add)
            self.n_ts += 1
            return T(o)
        o = self.fresh(out)
        op1 = OP.add if w.sg > 0 else OP.subtract
        self._veng().scalar_tensor_tensor(o, x.ap, float(a_eff), w.ap, OP.mult, op1)
        self.n_tt += 1
        return T(o)


def build_program():
    nc = bacc.Bacc("TRN2", target_bir_lowering=False, debug=False, num_devices=N_CORES)

    latent = nc.declare_dram_parameter("latent", [B, LAT], F32, isOutput=False)
    time_in = nc.declare_dram_parameter("time_in", [B, 1], F32, isOutput=False)
    w1 = nc.declare_dram_parameter("w1", [CIN, HID], F32, isOutput=False)
    b1 = nc.declare_dram_parameter("b1", [HID], F32, isOutput=False)
    w2 = nc.declare_dram_parameter("w2", [HID, HID], F32, isOutput=False)
    b2 = nc.declare_dram_parameter("b2", [HID], F32, isOutput=False)
    w3 = nc.declare_dram_parameter("w3", [HID, NJ], F32, isOutput=False)
    b3 = nc.declare_dram_parameter("b3", [NJ], F32, isOutput=False)
    out = nc.declare_dram_parameter("out", [B, NJ], F32, isOutput=True)

    with tile.TileContext(nc) as tc:
        with (
            tc.tile_pool(name="const", bufs=1) as cp,
            tc.tile_pool(name="work", bufs=2) as wp,
            tc.tile_pool(name="h1p", bufs=4) as h1p,
            tc.tile_pool(name="h2p", bufs=4) as h2p,
            tc.tile_pool(name="fk", bufs=48) as fkp,
            tc.tile_pool(name="ps_x", bufs=2, space="PSUM") as ps_x,
            tc.tile_pool(name="ps_h", bufs=4, space="PSUM") as ps_h,
            tc.tile_pool(name="ps_q", bufs=1, space="PSUM") as ps_q,
            tc.tile_pool(name="ps_qb", bufs=1, space="PSUM") as ps_qb,
        ):
            # ---- constants / weights ----
            w1s = cp.tile([CIN, HID], F32, tag="w1")
            nc.sync.dma_start(w1s[:], w1[:, :])
            w2s = cp.tile([2, P, HID], F32, tag="w2")
            nc.sync.dma_start(w2s[:], w2.rearrange("(k p) h -> k p h", p=P))
            w3s = cp.tile([2, P, NJ], F32, tag="w3")
            nc.sync.dma_start(w3s[:], w3.rearrange("(k p) j -> k p j", p=P))
            b1s = cp.tile([P, 2], F32, tag="b1")
            nc.sync.dma_start(b1s[:], b1.rearrange("(m p) -> p m", p=P))
            b2s = cp.tile([P, 2], F32, tag="b2")
            nc.sync.dma_start(b2s[:], b2.rearrange("(m p) -> p m", p=P))
            b3s = cp.tile([NJ, 1], F32, tag="b3")
            nc.sync.dma_start(b3s[:], b3.rearrange("j -> j 1"))

            ident = cp.tile([P, P], F32, tag="ident")
            make_identity(nc, ident[:])
            ident7 = cp.tile([NJ, NJ], F32, tag="ident7")
            make_identity(nc, ident7[:])

            # ---- load batch-major inputs ----
            x_bm = cp.tile([P, NCH, LAT], F32, tag="xbm")
            nc.sync.dma_start(x_bm[:], latent.rearrange("(p r) c -> p r c", p=P))
            t_bm = cp.tile([P, NCH], F32, tag="tbm")
            nc.sync.dma_start(t_bm[:], time_in.rearrange("(p r) c -> p (r c)", p=P))

            # ---- assemble gapped x (latent cols 0..15, time col 16) ----
            x_g = cp.tile([P, NCH, CIN], F32, tag="xg")
            nc.gpsimd.tensor_copy(x_g[:, :, 0:LAT], x_bm[:])
            nc.gpsimd.tensor_copy(x_g[:, :, LAT], t_bm[:])

            # ---- q in batch-major joint-major layout ----
            q_bm = cp.tile([P, NJ, NCH], F32, tag="qbm")
            pose = cp.tile([P, NCH, NJ], F32, tag="pose")

            for s in range(NSC):
                xt = wp.tile([CIN, SC], F32, tag="xt")
                # transpose x chunks: 4 per psum tile
                for g in range(CH_PER_SC // 4):
                    pxt = ps_x.tile([CIN, 512], F32, tag="pxt")
                    for jj in range(4):
                        ch = s * CH_PER_SC + g * 4 + jj
                        nc.tensor.transpose(
                            pxt[:, jj * P:(jj + 1) * P], x_g[:, ch, :], ident[:]
                        )
                    nc.scalar.copy(xt[:, g * 512:(g + 1) * 512], pxt[:])

                h1t = [wp.tile([P, SC], F32, tag=f"h1_{m}") for m in range(2)]
                h2t = [wp.tile([P, SC], F32, tag=f"h2_{m}") for m in range(2)]
                qt = wp.tile([NJ, SC], F32, tag="qt")

                for n in range(NN_PER_SC):
                    nsl = slice(n * 512, (n + 1) * 512)
                    # L1
                    for m in range(2):
                        psh = ps_h.tile([P, 512], F32, tag="psh")
                        nc.tensor.matmul(
                            psh[:],
                            _r(w1s[:, m * P:(m + 1) * P]),
                            _r(xt[:, nsl]),
                        )
                        nc.scalar.activation(
                            h1t[m][:, nsl], psh[:], AF.Relu,
                            bias=b1s[:, m:m + 1], scale=1.0,
                        )
                    # L2
                    for m in range(2):
                        psh = ps_h.tile([P, 512], F32, tag="psh")
                        nc.tensor.matmul(
                            psh[:],
                            _r(w2s[0, :, m * P:(m + 1) * P]),
                            _r(h1t[0][:, nsl]),
                            start=True, stop=False,
                        )
                        nc.tensor.matmul(
                            psh[:],
                            _r(w2s[1, :, m * P:(m + 1) * P]),
                            _r(h1t[1][:, nsl]),
                            start=False, stop=True,
                        )
                        nc.scalar.activation(
                            h2t[m][:, nsl], psh[:], AF.Relu,
                            bias=b2s[:, m:m + 1], scale=1.0,
                        )
                    # L3
                    psq = ps_q.tile([NJ, 512], F32, tag="psq")
                    nc.tensor.matmul(
                        psq[:], _r(w3s[0]), _r(h2t[0][:, nsl]),
                        start=True, stop=False,
                    )
                    nc.tensor.matmul(
                        psq[:], _r(w3s[1]), _r(h2t[1][:, nsl]),
                        start=False, stop=True,
                    )
                    nc.scalar.activation(
                        qt[:, nsl], psq[:], AF.Identity,
                        bias=b3s[:, 0:1], scale=1.0,
                    )

                # transpose q back to batch-major
                pqb = ps_qb.tile([P, CH_PER_SC * NJ], F32, tag="pqb")
                for j in range(CH_PER_SC):
                    nc.tensor.transpose(
                        pqb[:, j * NJ:(j + 1) * NJ], qt[:, j * P:(j + 1) * P],
                        ident7[:],
                    )
                nc.scalar.copy(
                    q_bm[:, :, s * CH_PER_SC:(s + 1) * CH_PER_SC],
                    pqb[:].rearrange("p (j i) -> p i j", i=NJ),
                )

            # ================= FK =================
            fkb = FKB(nc, fkp, NCH)

            # u = q0 + q1 (into q_bm slot 1); angles block = q_bm[:, 1:7, :]
            nc.vector.tensor_tensor(q_bm[:, 1, :], q_bm[:, 0, :], q_bm[:, 1, :], OP.add)
            ct = cp.tile([P, 6, NCH], F32, tag="ct")
            st = cp.tile([P, 6, NCH], F32, tag="st")
            nc.scalar.activation(ct[:], q_bm[:, 1:7, :], AF.Sin, bias=PI / 2, scale=1.0)
            nc.scalar.activation(st[:], q_bm[:, 1:7, :], AF.Sin, bias=0.0, scale=1.0)

            cu, su = ct[:, 0, :], st[:, 0, :]
            # state after joints 0..1
            rows = [
                [T(cu), ZERO, T(su, -1), ZERO],
                [T(su), ZERO, T(cu), ZERO],
                [ZERO, C(-1.0), ZERO, C(0.333)],
            ]

            for j in range(2, NJ):
                ctj, stj = ct[:, j - 1, :], st[:, j - 1, :]
                sa, a, d = DH_SA[j], DH_A[j], DH_D[j]
                last = j == NJ - 1
                new_rows = []
                for r in range(3):
                    x, y, z, w = rows[r]
                    n0 = fkb.lincomb(fkb.mul_trig(x, ctj), fkb.mul_trig(y, stj))
                    e = fkb.lincomb(fkb.mul_trig(x, stj), fkb.neg(fkb.mul_trig(y, ctj)))
                    y_new = z if sa > 0 else fkb.neg(z)
                    z_new = e if sa > 0 else fkb.neg(e)
                    out_ap = pose[:, :, r] if last else None
                    if d != 0.0:
                        wtmp = fkb.axpw(d, z, w)
                    else:
                        wtmp = w
                    w_new = fkb.axpw(a, n0, wtmp, out=out_ap)
                    if last and w_new.kind != "T":
                        # materialize constant position (cannot happen here)
                        nc.gpsimd.memset(pose[:, :, r], float(w_new.val))
                        w_new = T(pose[:, :, r])
                    new_rows.append([n0, y_new, z_new, w_new])
                rows = new_rows

            (r00, r01, r02, _), (r10, r11, r12, _), (r20, r21, r22, _) = rows

            tr = fkb.lincomb(fkb.lincomb(r00, r11), r22)
            # eta = sqrt(0.25*tr + 0.25)
            nc.scalar.activation(
                pose[:, :, 6], tr.ap, AF.Sqrt, bias=0.25, scale=0.25 * tr.sg
            )

            diag = [r00, r11, r22]
            gpairs = [(r21, r12), (r02, r20), (r10, r01)]
            for i in range(3):
                ui = fkb.axpw(2.0, diag[i], fkb.neg(tr))
                mag = fkb.fresh()
                nc.scalar.activation(
                    mag, ui.ap, AF.Sqrt, bias=0.25, scale=0.25 * ui.sg
                )
                ga, gb = gpairs[i]
                g = fkb.lincomb(ga, fkb.neg(gb))
                sg_t = fkb.fresh()
                nc.scalar.activation(sg_t, g.ap, AF.Sign, bias=0.0, scale=float(g.sg))
                nc.vector.tensor_tensor(pose[:, :, 3 + i], sg_t, mag, OP.mult)

            nc.sync.dma_start(out.rearrange("(p r) c -> p r c", p=P), pose[:])

    nc.compile()
    return nc


_PROG = None


def _get_prog():
    global _PROG
    if _PROG is None:
        _PROG = build_program()
    return _PROG


def kernel(latent_variable, time, W1, b1, W2, b2, W3, b3):
    nc = _get_prog()
    latent_variable = np.ascontiguousarray(latent_variable, dtype=np.float32)
    time = np.ascontiguousarray(time, dtype=np.float32)
    shared = {
        "w1": np.ascontiguousarray(W1, dtype=np.float32),
        "b1": np.ascontiguousarray(b1, dtype=np.float32),
        "w2": np.ascontiguousarray(W2, dtype=np.float32),
        "b2": np.ascontiguousarray(b2, dtype=np.float32),
        "w3": np.ascontiguousarray(W3, dtype=np.float32),
        "b3": np.ascontiguousarray(b3, dtype=np.float32),
    }
    in_maps = []
    for c in range(N_CORES):
        sl = slice(c * B, (c + 1) * B)
        m = {"latent": latent_variable[sl], "time_in": time[sl]}
        m.update(shared)
        in_maps.append(m)
    res = run_bass_kernel_spmd(nc, in_maps, list(range(N_CORES)))
    return np.concatenate(
        [res.results[c]["out"] for c in range(N_CORES)], axis=0
    ).astype(np.float32)


# revision 10
# speedup vs baseline: 1.1881x; 1.1881x over previous
import sys

sys.path.insert(0, "/opt/trn_rl_repo")

import math

import numpy as np

import concourse.bass as bass
import concourse.mybir as mybir
import concourse.tile as tile
from concourse import bacc
from concourse.bass_utils import run_bass_kernel_spmd
from concourse.masks import make_identity

F32 = mybir.dt.float32
F32R = mybir.dt.float32r
AF = mybir.ActivationFunctionType
OP = mybir.AluOpType

N_CORES = 8
B_TOTAL = 131072
B = B_TOTAL // N_CORES  # 16384 rows per core
P = 128
NCH = B // P  # 128 chunks of 128 rows
LAT = 16
CIN = 17
HID = 256
NJ = 7

SC = 2048          # super-chunk width (b' columns)
NSC = B // SC      # 8 super-chunks
CH_PER_SC = SC // P  # 16 x-chunks per super-chunk
NN_PER_SC = SC // 512  # 4 N-chunks of 512

PI = math.pi

# Franka DH constants
DH_A = [0.0, 0.0, 0.0, 0.0825, -0.0825, 0.0, 0.088]
DH_D = [0.333, 0.0, 0.316, 0.0, 0.384, 0.0, 0.0]
DH_SA = [0, -1, 1, 1, -1, 1, 1]  # sin(alpha), exact

USE_F32R = True


def _r(ap):
    """View an f32 AP as float32r for full-rate PE matmuls."""
    if USE_F32R:
        return ap.bitcast(F32R)
    return ap


# ----------------------------------------------------------------------------
# FK symbolic builder: entries are Zero, Const, or Tile(ap, sigma)
# ----------------------------------------------------------------------------
class E:
    __slots__ = ("kind", "val", "ap", "sg")

    def __init__(self, kind, val=0.0, ap=None, sg=1):
        self.kind = kind  # 'Z' | 'C' | 'T'
        self.val = val
        self.ap = ap
        self.sg = sg


ZERO = E("Z")


def C(v):
    return E("C", val=v)


def T(ap, sg=1):
    return E("T", ap=ap, sg=sg)


class FKB:
    """Emits bass ops for the FK chain with compile-time constant folding."""

    def __init__(self, nc, pool, nb):
        self.nc = nc
        self.pool = pool
        self.nb = nb
        self.n_tt = 0  # op counters
        self.n_ts = 0
        self.rr = 0

    def fresh(self, out=None):
        if out is not None:
            return out
        self.n_fresh = getattr(self, "n_fresh", 0) + 1
        return self.pool.tile([P, self.nb], F32, tag="fk", name=f"fkt{self.n_fresh}")[:]

    def _veng(self):
        # round-robin heavy 2-input ops between DVE and GPSIMD
        self.rr += 1
        return self.nc.vector if (self.rr % 2) else self.nc.gpsimd

    def mul_trig(self, x, trig_ap):
        """entry * trig tile -> entry"""
        nc = self.nc
        if x.kind == "Z":
            return ZERO
        if x.kind == "C":
            o = self.fresh()
            nc.gpsimd.tensor_scalar_mul(o, trig_ap, float(x.val))
            self.n_ts += 1
            return T(o)
        o = self.fresh()
        self._veng().tensor_tensor(o, x.ap, trig_ap, OP.mult)
        self.n_tt += 1
        return T(o, x.sg)

    def lincomb(self, a, b, out=None):
        """a + b (entries with signs) -> entry (one TT op when both tiles)."""
        nc = self.nc
        if a.kind == "Z":
            if out is not None and b.kind == "T":
                nc.gpsimd.tensor_scalar_mul(out, b.ap, float(b.sg))
                self.n_ts += 1
                return T(out)
            return b
        if b.kind == "Z":
            if out is not None and a.kind == "T":
                nc.gpsimd.tensor_scalar_mul(out, a.ap, float(a.sg))
                self.n_ts += 1
                return T(out)
            return a
        assert a.kind == "T" and b.kind == "T"
        o = self.fresh(out)
        eng = self._veng()
        if a.sg > 0 and b.sg > 0:
            eng.tensor_tensor(o, a.ap, b.ap, OP.add)
            sg = 1
        elif a.sg > 0 and b.sg < 0:
            eng.tensor_tensor(o, a.ap, b.ap, OP.subtract)
            sg = 1
        elif a.sg < 0 and b.sg > 0:
            eng.tensor_tensor(o, b.ap, a.ap, OP.subtract)
            sg = 1
        else:
            eng.tensor_tensor(o, a.ap, b.ap, OP.add)
            sg = -1
        self.n_tt += 1
        return T(o, sg)

    def neg(self, a):
        if a.kind == "Z":
            return ZERO
        if a.kind == "C":
            return C(-a.val)
        return T(a.ap, -a.sg)

    def axpw(self, a_const, x, w, out=None):
        """a_const * x + w -> entry with sigma=+ (true value)."""
        nc = self.nc
        if x.kind == "Z" or a_const == 0.0:
            # result = w
            if w.kind == "T" and out is not None:
                nc.gpsimd.tensor_scalar_mul(out, w.ap, float(w.sg))
                self.n_ts += 1
                return T(out)
            return w
        if x.kind == "C":
            cv = a_const * x.val
            if w.kind == "Z":
                return C(cv)
            if w.kind == "C":
                return C(cv + w.val)
            o = self.fresh(out)
            nc.gpsimd.tensor_scalar(o, w.ap, float(w.sg), cv, OP.mult, OP.add)
            self.n_ts += 1
            return T(o)
        a_eff = a_const * x.sg
        if w.kind == "Z":
            o = self.fresh(out)
            nc.gpsimd.tensor_scalar_mul(o, x.ap, float(a_eff))
            self.n_ts += 1
            return T(o)
        if w.kind == "C":
            o = self.fresh(out)
            nc.gpsimd.tensor_scalar(o, x.ap, float(a_eff), float(w.val), OP.mult, OP.add)
            self.n_ts += 1
            return T(o)
        o = self.fresh(out)
        op1 = OP.add if w.sg > 0 else OP.subtract
        self.nc.vector.scalar_tensor_tensor(o, x.ap, float(a_eff), w.ap, OP.mult, op1)
        self.n_tt += 1
        return T(o)


def build_program():
    nc = bacc.Bacc("TRN2", target_bir_lowering=False, debug=False, num_devices=N_CORES)

    latent = nc.declare_dram_parameter("latent", [B, LAT], F32, isOutput=False)
    time_in = nc.declare_dram_parameter("time_in", [B, 1], F32, isOutput=False)
    w1 = nc.declare_dram_parameter("w1", [CIN, HID], F32, isOutput=False)
    b1 = nc.declare_dram_parameter("b1", [HID], F32, isOutput=False)
    w2 = nc.declare_dram_parameter("w2", [HID, HID], F32, isOutput=False)
    b2 = nc.declare_dram_parameter("b2", [HID], F32, isOutput=False)
    w3 = nc.declare_dram_parameter("w3", [HID, NJ], F32, isOutput=False)
    b3 = nc.declare_dram_parameter("b3", [NJ], F32, isOutput=False)
    out = nc.declare_dram_parameter("out", [B, NJ], F32, isOutput=True)

    with tile.TileContext(nc) as tc:
        with (
            tc.tile_pool(name="const", bufs=1) as cp,
            tc.tile_pool(name="work", bufs=2) as wp,
            tc.tile_pool(name="h1p", bufs=4) as h1p,
            tc.tile_pool(name="h2p", bufs=4) as h2p,
            tc.tile_pool(name="fk", bufs=48) as fkp,
            tc.tile_pool(name="ps_x", bufs=2, space="PSUM") as ps_x,
            tc.tile_pool(name="ps_h", bufs=4, space="PSUM") as ps_h,
            tc.tile_pool(name="ps_q", bufs=1, space="PSUM") as ps_q,
            tc.tile_pool(name="ps_qb", bufs=1, space="PSUM") as ps_qb,
        ):
            # ---- constants / weights ----
            w1s = cp.tile([CIN, HID], F32R, tag="w1")
            nc.gpsimd.dma_start(w1s[:], w1[:, :])
            w2s = cp.tile([P, 2, HID], F32R, tag="w2")
            nc.gpsimd.dma_start(w2s[:], w2.rearrange("(k p) h -> p k h", p=P))
            w3s = cp.tile([P, 2, NJ], F32R, tag="w3")
            nc.gpsimd.dma_start(w3s[:], w3.rearrange("(k p) j -> p k j", p=P))
            b1s = cp.tile([P, 2], F32, tag="b1")
            nc.sync.dma_start(b1s[:], b1.rearrange("(m p) -> p m", p=P))
            b2s = cp.tile([P, 2], F32, tag="b2")
            nc.sync.dma_start(b2s[:], b2.rearrange("(m p) -> p m", p=P))
            b3s = cp.tile([NJ, 1], F32, tag="b3")
            nc.sync.dma_start(b3s[:], b3.rearrange("(j one) -> j one", one=1))

            halfpi = cp.tile([P, 1], F32, tag="halfpi")
            nc.gpsimd.memset(halfpi[:], PI / 2)
            quarter = cp.tile([P, 1], F32, tag="quarter")
            nc.gpsimd.memset(quarter[:], 0.25)
            zerocol = cp.tile([P, 1], F32, tag="zerocol")
            nc.gpsimd.memset(zerocol[:], 0.0)

            ident_f = cp.tile([P, P], F32, tag="ident_f")
            make_identity(nc, ident_f[:])
            ident = cp.tile([P, P], F32R, tag="ident")
            nc.gpsimd.tensor_copy(ident[:], ident_f[:])
            ident7 = cp.tile([NJ, NJ], F32, tag="ident7")
            make_identity(nc, ident7[:])

            # ---- load batch-major inputs ----
            x_bm = cp.tile([P, NCH, LAT], F32, tag="xbm")
            nc.sync.dma_start(x_bm[:], latent.rearrange("(p r) c -> p r c", p=P))
            t_bm = cp.tile([P, NCH], F32, tag="tbm")
            nc.sync.dma_start(t_bm[:], time_in.rearrange("(p r) c -> p (r c)", p=P))

            # ---- assemble gapped x (latent cols 0..15, time col 16) ----
            x_g = cp.tile([P, NCH, CIN], F32R, tag="xg")
            nc.gpsimd.tensor_copy(x_g[:, :, 0:LAT], x_bm[:])
            nc.gpsimd.tensor_copy(x_g[:, :, LAT], t_bm[:])

            # ---- q in batch-major joint-major layout ----
            q_bm = cp.tile([P, NJ, NCH], F32, tag="qbm")
            pose = cp.tile([P, NCH, NJ], F32, tag="pose")

            for s in range(NSC):
                xt = wp.tile([CIN, SC], F32R, tag="xt")
                # transpose x chunks: 4 per psum tile
                for g in range(CH_PER_SC // 4):
                    pxt = ps_x.tile([CIN, 512], F32R, tag="pxt")
                    for jj in range(4):
                        ch = s * CH_PER_SC + g * 4 + jj
                        nc.tensor.transpose(
                            pxt[:, jj * P:(jj + 1) * P], x_g[:, ch, :], ident[:]
                        )
                    nc.scalar.copy(xt[:, g * 512:(g + 1) * 512], pxt[:])

                h1t = [wp.tile([P, SC], F32R, tag=f"h1_{m}", name=f"h1t{m}") for m in range(2)]
                h2t = [wp.tile([P, SC], F32R, tag=f"h2_{m}", name=f"h2t{m}") for m in range(2)]
                qt = wp.tile([NJ, SC], F32, tag="qt")

                for n in range(NN_PER_SC):
                    nsl = slice(n * 512, (n + 1) * 512)
                    # L1
                    for m in range(2):
                        psh = ps_h.tile([P, 512], F32, tag="psh")
                        nc.tensor.matmul(
                            psh[:],
                            w1s[:, m * P:(m + 1) * P],
                            xt[:, nsl],
                        )
                        nc.vector.tensor_scalar(
                            h1t[m][:, nsl], psh[:], b1s[:, m:m + 1], 0.0,
                            OP.add, OP.max,
                        )
                    # L2
                    for m in range(2):
                        psh = ps_h.tile([P, 512], F32, tag="psh")
                        nc.tensor.matmul(
                            psh[:],
                            w2s[:, 0, m * P:(m + 1) * P],
                            h1t[0][:, nsl],
                            start=True, stop=False,
                        )
                        nc.tensor.matmul(
                            psh[:],
                            w2s[:, 1, m * P:(m + 1) * P],
                            h1t[1][:, nsl],
                            start=False, stop=True,
                        )
                        if (n + m) % 2 == 0:
                            nc.scalar.activation(
                                h2t[m][:, nsl], psh[:], AF.Relu,
                                bias=b2s[:, m:m + 1], scale=1.0,
                            )
                        else:
                            nc.vector.tensor_scalar(
                                h2t[m][:, nsl], psh[:], b2s[:, m:m + 1], 0.0,
                                OP.add, OP.max,
                            )
                    # L3
                    psq = ps_q.tile([NJ, 512], F32, tag="psq")
                    nc.tensor.matmul(
                        psq[:], w3s[:, 0, :], h2t[0][:, nsl],
                        start=True, stop=False,
                    )
                    nc.tensor.matmul(
                        psq[:], w3s[:, 1, :], h2t[1][:, nsl],
                        start=False, stop=True,
                    )
                    nc.scalar.activation(
                        qt[:, nsl], psq[:], AF.Identity,
                        bias=b3s[:, 0:1], scale=1.0,
                    )

                # transpose q back to batch-major
                pqb = ps_qb.tile([P, CH_PER_SC * NJ], F32, tag="pqb")
                for j in range(CH_PER_SC):
                    nc.tensor.transpose(
                        pqb[:, j * NJ:(j + 1) * NJ], qt[:, j * P:(j + 1) * P],
                        ident7[:],
                    )
                nc.scalar.copy(
                    q_bm[:, :, s * CH_PER_SC:(s + 1) * CH_PER_SC],
                    pqb[:].rearrange("p (j i) -> p i j", i=NJ),
                )

            # ================= FK =================
            fkb = FKB(nc, fkp, NCH)

            # u = q0 + q1 (into q_bm slot 1); angles block = q_bm[:, 1:7, :]
            nc.vector.tensor_tensor(q_bm[:, 1, :], q_bm[:, 0, :], q_bm[:, 1, :], OP.add)
            ct = cp.tile([P, 6, NCH], F32, tag="ct")
            st = cp.tile([P, 6, NCH], F32, tag="st")
            nc.scalar.activation(ct[:], q_bm[:, 1:7, :], AF.Sin, bias=halfpi[:], scale=1.0)
            nc.scalar.activation(st[:], q_bm[:, 1:7, :], AF.Sin, bias=zerocol[:], scale=1.0)

            cu, su = ct[:, 0, :], st[:, 0, :]
            # state after joints 0..1
            rows = [
                [T(cu), ZERO, T(su, -1), ZERO],
                [T(su), ZERO, T(cu), ZERO],
                [ZERO, C(-1.0), ZERO, C(0.333)],
            ]

            for j in range(2, NJ):
                ctj, stj = ct[:, j - 1, :], st[:, j - 1, :]
                sa, a, d = DH_SA[j], DH_A[j], DH_D[j]
                last = j == NJ - 1
                new_rows = []
                for r in range(3):
                    x, y, z, w = rows[r]
                    n0 = fkb.lincomb(fkb.mul_trig(x, ctj), fkb.mul_trig(y, stj))
                    e = fkb.lincomb(fkb.mul_trig(x, stj), fkb.neg(fkb.mul_trig(y, ctj)))
                    y_new = z if sa > 0 else fkb.neg(z)
                    z_new = e if sa > 0 else fkb.neg(e)
                    out_ap = pose[:, :, r] if last else None
                    if d != 0.0:
                        wtmp = fkb.axpw(d, z, w)
                    else:
                        wtmp = w
                    w_new = fkb.axpw(a, n0, wtmp, out=out_ap)
                    if last and w_new.kind != "T":
                        # materialize constant position (cannot happen here)
                        nc.gpsimd.memset(pose[:, :, r], float(w_new.val))
                        w_new = T(pose[:, :, r])
                    new_rows.append([n0, y_new, z_new, w_new])
                rows = new_rows

            (r00, r01, r02, _), (r10, r11, r12, _), (r20, r21, r22, _) = rows

            tr = fkb.lincomb(fkb.lincomb(r00, r11), r22)
            # eta = sqrt(0.25*tr + 0.25)
            nc.scalar.activation(
                pose[:, :, 6], tr.ap, AF.Sqrt, bias=quarter[:], scale=0.25 * tr.sg
            )

            diag = [r00, r11, r22]
            gpairs = [(r21, r12), (r02, r20), (r10, r01)]
            for i in range(3):
                ui = fkb.axpw(2.0, diag[i], fkb.neg(tr))
                mag = fkb.fresh()
                nc.scalar.activation(
                    mag, ui.ap, AF.Sqrt, bias=quarter[:], scale=0.25 * ui.sg
                )
                ga, gb = gpairs[i]
                g = fkb.lincomb(ga, fkb.neg(gb))
                sg_t = fkb.fresh()
                nc.scalar.activation(sg_t, g.ap, AF.Sign, bias=zerocol[:], scale=float(g.sg))
                nc.vector.tensor_tensor(pose[:, :, 3 + i], sg_t, mag, OP.mult)

            nc.sync.dma_start(out.rearrange("(p r) c -> p r c", p=P), pose[:])

    nc.compile()
    return nc


_PROG = None


def _get_prog():
    global _PROG
    if _PROG is None:
        _PROG = build_program()
    return _PROG


def kernel(latent_variable, time, W1, b1, W2, b2, W3, b3):
    nc = _get_prog()
    latent_variable = np.ascontiguousarray(latent_variable, dtype=np.float32)
    time = np.ascontiguousarray(time, dtype=np.float32)
    shared = {
        "w1": np.ascontiguousarray(W1, dtype=np.float32),
        "b1": np.ascontiguousarray(b1, dtype=np.float32),
        "w2": np.ascontiguousarray(W2, dtype=np.float32),
        "b2": np.ascontiguousarray(b2, dtype=np.float32),
        "w3": np.ascontiguousarray(W3, dtype=np.float32),
        "b3": np.ascontiguousarray(b3, dtype=np.float32),
    }
    in_maps = []
    for c in range(N_CORES):
        sl = slice(c * B, (c + 1) * B)
        m = {"latent": latent_variable[sl], "time_in": time[sl]}
        m.update(shared)
        in_maps.append(m)
    res = run_bass_kernel_spmd(nc, in_maps, list(range(N_CORES)), **_RUN_KWARGS)
    global LAST_RESULT
    LAST_RESULT = res
    return np.concatenate(
        [res.results[c]["out"] for c in range(N_CORES)], axis=0
    ).astype(np.float32)


LAST_RESULT = None
_RUN_KWARGS = {}


# revision 13
# speedup vs baseline: 1.2658x; 1.0653x over previous
import sys

sys.path.insert(0, "/opt/trn_rl_repo")

import math

import numpy as np

import concourse.bass as bass
import concourse.mybir as mybir
import concourse.tile as tile
from concourse import bacc
from concourse.bass_utils import run_bass_kernel_spmd
from concourse.masks import make_identity

F32 = mybir.dt.float32
F32R = mybir.dt.float32r
AF = mybir.ActivationFunctionType
OP = mybir.AluOpType

N_CORES = 8
B_TOTAL = 131072
B = B_TOTAL // N_CORES  # 16384 rows per core
P = 128
NCH = B // P  # 128 chunks of 128 rows
LAT = 16
CIN = 17
HID = 256
NJ = 7

SC = 2048          # super-chunk width (b' columns)
NSC = B // SC      # 8 super-chunks
CH_PER_SC = SC // P  # 16 x-chunks per super-chunk
NN_PER_SC = SC // 512  # 4 N-chunks of 512

PI = math.pi

# Franka DH constants
DH_A = [0.0, 0.0, 0.0, 0.0825, -0.0825, 0.0, 0.088]
DH_D = [0.333, 0.0, 0.316, 0.0, 0.384, 0.0, 0.0]
DH_SA = [0, -1, 1, 1, -1, 1, 1]  # sin(alpha), exact

USE_F32R = True


def _r(ap):
    """View an f32 AP as float32r for full-rate PE matmuls."""
    if USE_F32R:
        return ap.bitcast(F32R)
    return ap


# ----------------------------------------------------------------------------
# FK symbolic builder: entries are Zero, Const, or Tile(ap, sigma)
# ----------------------------------------------------------------------------
class E:
    __slots__ = ("kind", "val", "ap", "sg")

    def __init__(self, kind, val=0.0, ap=None, sg=1):
        self.kind = kind  # 'Z' | 'C' | 'T'
        self.val = val
        self.ap = ap
        self.sg = sg


ZERO = E("Z")


def C(v):
    return E("C", val=v)


def T(ap, sg=1):
    return E("T", ap=ap, sg=sg)


class FKB:
    """Emits bass ops for the FK chain with compile-time constant folding."""

    def __init__(self, nc, pool, nb):
        self.nc = nc
        self.pool = pool
        self.nb = nb
        self.n_tt = 0  # op counters
        self.n_ts = 0
        self.rr = 0

    def fresh(self, out=None):
        if out is not None:
            return out
        self.n_fresh = getattr(self, "n_fresh", 0) + 1
        return self.pool.tile([P, self.nb], F32, tag="fk", name=f"fkt{self.n_fresh}")[:]

    def _veng(self):
        # round-robin heavy 2-input ops between DVE and GPSIMD
        self.rr += 1
        return self.nc.vector if (self.rr % 2) else self.nc.gpsimd

    def mul_trig(self, x, trig_ap):
        """entry * trig tile -> entry"""
        nc = self.nc
        if x.kind == "Z":
            return ZERO
        if x.kind == "C":
            o = self.fresh()
            nc.gpsimd.tensor_scalar_mul(o, trig_ap, float(x.val))
            self.n_ts += 1
            return T(o)
        o = self.fresh()
        self._veng().tensor_tensor(o, x.ap, trig_ap, OP.mult)
        self.n_tt += 1
        return T(o, x.sg)

    def lincomb(self, a, b, out=None):
        """a + b (entries with signs) -> entry (one TT op when both tiles)."""
        nc = self.nc
        if a.kind == "Z":
            if out is not None and b.kind == "T":
                nc.gpsimd.tensor_scalar_mul(out, b.ap, float(b.sg))
                self.n_ts += 1
                return T(out)
            return b
        if b.kind == "Z":
            if out is not None and a.kind == "T":
                nc.gpsimd.tensor_scalar_mul(out, a.ap, float(a.sg))
                self.n_ts += 1
                return T(out)
            return a
        assert a.kind == "T" and b.kind == "T"
        o = self.fresh(out)
        eng = self._veng()
        if a.sg > 0 and b.sg > 0:
            eng.tensor_tensor(o, a.ap, b.ap, OP.add)
            sg = 1
        elif a.sg > 0 and b.sg < 0:
            eng.tensor_tensor(o, a.ap, b.ap, OP.subtract)
            sg = 1
        elif a.sg < 0 and b.sg > 0:
            eng.tensor_tensor(o, b.ap, a.ap, OP.subtract)
            sg = 1
        else:
            eng.tensor_tensor(o, a.ap, b.ap, OP.add)
            sg = -1
        self.n_tt += 1
        return T(o, sg)

    def neg(self, a):
        if a.kind == "Z":
            return ZERO
        if a.kind == "C":
            return C(-a.val)
        return T(a.ap, -a.sg)

    def axpw(self, a_const, x, w, out=None):
        """a_const * x + w -> entry with sigma=+ (true value)."""
        nc = self.nc
        if x.kind == "Z" or a_const == 0.0:
            # result = w
            if w.kind == "T" and out is not None:
                nc.gpsimd.tensor_scalar_mul(out, w.ap, float(w.sg))
                self.n_ts += 1
                return T(out)
            return w
        if x.kind == "C":
            cv = a_const * x.val
            if w.kind == "Z":
                return C(cv)
            if w.kind == "C":
                return C(cv + w.val)
            o = self.fresh(out)
            nc.gpsimd.tensor_scalar(o, w.ap, float(w.sg), cv, OP.mult, OP.add)
            self.n_ts += 1
            return T(o)
        a_eff = a_const * x.sg
        if w.kind == "Z":
            o = self.fresh(out)
            nc.gpsimd.tensor_scalar_mul(o, x.ap, float(a_eff))
            self.n_ts += 1
            return T(o)
        if w.kind == "C":
            o = self.fresh(out)
            nc.gpsimd.tensor_scalar(o, x.ap, float(a_eff), float(w.val), OP.mult, OP.add)
            self.n_ts += 1
            return T(o)
        o = self.fresh(out)
        op1 = OP.add if w.sg > 0 else OP.subtract
        self.nc.vector.scalar_tensor_tensor(o, x.ap, float(a_eff), w.ap, OP.mult, op1)
        self.n_tt += 1
        return T(o)


def build_program():
    nc = bacc.Bacc("TRN2", target_bir_lowering=False, debug=False, num_devices=N_CORES)

    latent = nc.declare_dram_parameter("latent", [B, LAT], F32, isOutput=False)
    time_in = nc.declare_dram_parameter("time_in", [B, 1], F32, isOutput=False)
    w1 = nc.declare_dram_parameter("w1", [CIN, HID], F32, isOutput=False)
    b1 = nc.declare_dram_parameter("b1", [HID], F32, isOutput=False)
    w2 = nc.declare_dram_parameter("w2", [HID, HID], F32, isOutput=False)
    b2 = nc.declare_dram_parameter("b2", [HID], F32, isOutput=False)
    w3 = nc.declare_dram_parameter("w3", [HID, NJ], F32, isOutput=False)
    b3 = nc.declare_dram_parameter("b3", [NJ], F32, isOutput=False)
    out = nc.declare_dram_parameter("out", [B, NJ], F32, isOutput=True)

    with tile.TileContext(nc) as tc:
        with (
            tc.tile_pool(name="const", bufs=1) as cp,
            tc.tile_pool(name="work", bufs=2) as wp,
            tc.tile_pool(name="h1p", bufs=4) as h1p,
            tc.tile_pool(name="h2p", bufs=4) as h2p,
            tc.tile_pool(name="fk", bufs=48) as fkp,
            tc.tile_pool(name="ps_x", bufs=1, space="PSUM") as ps_x,
            tc.tile_pool(name="ps_qb", bufs=1, space="PSUM") as ps_qb,
            tc.tile_pool(name="ps_h", bufs=4, space="PSUM") as ps_h,
            tc.tile_pool(name="ps_q", bufs=2, space="PSUM") as ps_q,
        ):
            # ---- constants / weights ----
            w1r = cp.tile([P, HID], F32R, tag="w1")
            for g in range(4):
                nc.gpsimd.dma_start(w1r[32 * g:32 * g + CIN, :], w1[:, :])
            w2s = cp.tile([P, 2, HID], F32R, tag="w2")
            nc.gpsimd.dma_start(w2s[:], w2.rearrange("(k p) h -> p k h", p=P))
            w3s = cp.tile([P, 2, NJ], F32R, tag="w3")
            nc.gpsimd.dma_start(w3s[:], w3.rearrange("(k p) j -> p k j", p=P))
            b1s = cp.tile([P, 2], F32, tag="b1")
            nc.sync.dma_start(b1s[:], b1.rearrange("(m p) -> p m", p=P))
            b2s = cp.tile([P, 2], F32, tag="b2")
            nc.sync.dma_start(b2s[:], b2.rearrange("(m p) -> p m", p=P))
            b3s = cp.tile([NJ, 1], F32, tag="b3")
            nc.sync.dma_start(b3s[:], b3.rearrange("(j one) -> j one", one=1))

            halfpi = cp.tile([P, 1], F32, tag="halfpi")
            nc.gpsimd.memset(halfpi[:], PI / 2)
            quarter = cp.tile([P, 1], F32, tag="quarter")
            nc.gpsimd.memset(quarter[:], 0.25)
            zerocol = cp.tile([P, 1], F32, tag="zerocol")
            nc.gpsimd.memset(zerocol[:], 0.0)

            ident_f = cp.tile([P, P], F32, tag="ident_f")
            make_identity(nc, ident_f[:])
            ident = cp.tile([P, P], F32R, tag="ident")
            nc.gpsimd.tensor_copy(ident[:], ident_f[:])


            # ---- load batch-major inputs ----
            x_bm = cp.tile([P, NCH, LAT], F32, tag="xbm")
            nc.sync.dma_start(x_bm[:], latent.rearrange("(p r) c -> p r c", p=P))
            t_bm = cp.tile([P, NCH], F32, tag="tbm")
            nc.sync.dma_start(t_bm[:], time_in.rearrange("(p r) c -> p (r c)", p=P))

            # ---- assemble gapped x (latent cols 0..15, time col 16) ----
            x_g = cp.tile([P, NCH // 4, 4, 32], F32R, tag="xg")
            nc.gpsimd.tensor_copy(
                x_g[:, :, :, 0:LAT],
                x_bm[:].rearrange("p (q g) c -> p q g c", g=4),
            )
            nc.gpsimd.tensor_copy(
                x_g[:, :, :, LAT],
                t_bm[:].rearrange("p (q g) -> p q g", g=4),
            )

            # ---- q in batch-major joint-major layout ----
            q_bm = cp.tile([P, NJ, NCH], F32, tag="qbm")
            pose = cp.tile([P, NCH, NJ], F32, tag="pose")

            for s in range(NSC):
                xt = wp.tile([P, 512], F32R, tag="xt")
                pxt = ps_x.tile([P, 512], F32R, tag="pxt")
                for q in range(4):
                    nc.tensor.transpose(
                        pxt[:, q * P:(q + 1) * P],
                        x_g[:, s * 4 + q, :, :], ident[:],
                    )
                nc.scalar.copy(xt[:], pxt[:])

                h1t = [wp.tile([P, SC], F32R, tag=f"h1_{m}", name=f"h1t{m}") for m in range(2)]
                h2t = [wp.tile([P, SC], F32R, tag=f"h2_{m}", name=f"h2t{m}") for m in range(2)]
                qt = wp.tile([NJ, SC], F32, tag="qt")

                # L1: 4-way row-tiled over partition strips
                for m in range(2):
                    for g in range(4):
                        psh = ps_h.tile([P, 512], F32, tag="psh")
                        nc.tensor.matmul(
                            psh[:],
                            w1r[32 * g:32 * g + CIN, m * P:(m + 1) * P],
                            xt[32 * g:32 * g + CIN, :],
                            tile_position=(32 * g, 0),
                        )
                        dst = h1t[m][:, g * 512:(g + 1) * 512]
                        if (m + g) % 2 == 0:
                            nc.vector.tensor_scalar(
                                dst, psh[:], b1s[:, m:m + 1], 0.0, OP.add, OP.max
                            )
                        else:
                            nc.scalar.activation(
                                dst, psh[:], AF.Relu, bias=b1s[:, m:m + 1], scale=1.0
                            )

                # L2
                for n in range(NN_PER_SC):
                    nsl = slice(n * 512, (n + 1) * 512)
                    for m in range(2):
                        psh = ps_h.tile([P, 512], F32, tag="psh")
                        nc.tensor.matmul(
                            psh[:],
                            w2s[:, 0, m * P:(m + 1) * P],
                            h1t[0][:, nsl],
                            start=True, stop=False,
                        )
                        nc.tensor.matmul(
                            psh[:],
                            w2s[:, 1, m * P:(m + 1) * P],
                            h1t[1][:, nsl],
                            start=False, stop=True,
                        )
                        dst = h2t[m][:, nsl]
                        if (n + m) % 2 == 0:
                            nc.scalar.activation(
                                dst, psh[:], AF.Relu, bias=b2s[:, m:m + 1], scale=1.0
                            )
                        else:
                            nc.vector.tensor_scalar(
                                dst, psh[:], b2s[:, m:m + 1], 0.0, OP.add, OP.max
                            )

                # L3: feature-major q^T [7, 512] per strip-block
                for g in range(4):
                    gsl = slice(g * 512, (g + 1) * 512)
                    psq = ps_q.tile([NJ, 512], F32, tag="psq")
                    nc.tensor.matmul(
                        psq[:], w3s[:, 0, :], h2t[0][:, gsl],
                        start=True, stop=False,
                    )
                    nc.tensor.matmul(
                        psq[:], w3s[:, 1, :], h2t[1][:, gsl],
                        start=False, stop=True,
                    )
                    nc.scalar.activation(
                        qt[:, gsl], psq[:], AF.Identity, bias=b3s[:, 0:1], scale=1.0
                    )

                # scatter q^T strips across partitions via DMA, then quad-transpose
                qts = wp.tile([P, 512], F32, tag="qts")
                for g in range(4):
                    nc.sync.dma_start(
                        qts[32 * g:32 * g + NJ, :], qt[:, g * 512:(g + 1) * 512]
                    )
                pqb = ps_qb.tile([P, 512], F32, tag="pqb")
                for q in range(4):
                    nc.tensor.transpose(
                        pqb[:, q * P:(q + 1) * P],
                        qts[:, q * P:(q + 1) * P], ident_f[:],
                    )
                nc.scalar.copy(
                    q_bm[:, :, s * CH_PER_SC:(s + 1) * CH_PER_SC].rearrange(
                        "p i (q g) -> p i q g", g=4
                    ),
                    pqb[:].rearrange("p (q g i) -> p i q g", q=4, g=4)[:, 0:NJ, :, :],
                )

            # ================= FK =================
            fkb = FKB(nc, fkp, NCH)

            # u = q0 + q1 (into q_bm slot 1); angles block = q_bm[:, 1:7, :]
            nc.vector.tensor_tensor(q_bm[:, 1, :], q_bm[:, 0, :], q_bm[:, 1, :], OP.add)
            ct = cp.tile([P, 6, NCH], F32, tag="ct")
            st = cp.tile([P, 6, NCH], F32, tag="st")
            nc.scalar.activation(ct[:], q_bm[:, 1:7, :], AF.Sin, bias=halfpi[:], scale=1.0)
            nc.scalar.activation(st[:], q_bm[:, 1:7, :], AF.Sin, bias=zerocol[:], scale=1.0)

            cu, su = ct[:, 0, :], st[:, 0, :]
            # state after joints 0..1
            rows = [
                [T(cu), ZERO, T(su, -1), ZERO],
                [T(su), ZERO, T(cu), ZERO],
                [ZERO, C(-1.0), ZERO, C(0.333)],
            ]

            for j in range(2, NJ):
                ctj, stj = ct[:, j - 1, :], st[:, j - 1, :]
                sa, a, d = DH_SA[j], DH_A[j], DH_D[j]
                last = j == NJ - 1
                new_rows = []
                for r in range(3):
                    x, y, z, w = rows[r]
                    n0 = fkb.lincomb(fkb.mul_trig(x, ctj), fkb.mul_trig(y, stj))
                    e = fkb.lincomb(fkb.mul_trig(x, stj), fkb.neg(fkb.mul_trig(y, ctj)))
                    y_new = z if sa > 0 else fkb.neg(z)
                    z_new = e if sa > 0 else fkb.neg(e)
                    out_ap = pose[:, :, r] if last else None
                    if d != 0.0:
                        wtmp = fkb.axpw(d, z, w)
                    else:
                        wtmp = w
                    w_new = fkb.axpw(a, n0, wtmp, out=out_ap)
                    if last and w_new.kind != "T":
                        # materialize constant position (cannot happen here)
                        nc.gpsimd.memset(pose[:, :, r], float(w_new.val))
                        w_new = T(pose[:, :, r])
                    new_rows.append([n0, y_new, z_new, w_new])
                rows = new_rows

            (r00, r01, r02, _), (r10, r11, r12, _), (r20, r21, r22, _) = rows

            tr = fkb.lincomb(fkb.lincomb(r00, r11), r22)
            # eta = sqrt(0.25*tr + 0.25)
            nc.scalar.activation(
                pose[:, :, 6], tr.ap, AF.Sqrt, bias=quarter[:], scale=0.25 * tr.sg
            )

            diag = [r00, r11, r22]
            gpairs = [(r21, r12), (r02, r20), (r10, r01)]
            for i in range(3):
                ui = fkb.axpw(2.0, diag[i], fkb.neg(tr))
                mag = fkb.fresh()
                nc.scalar.activation(
                    mag, ui.ap, AF.Sqrt, bias=quarter[:], scale=0.25 * ui.sg
                )
                ga, gb = gpairs[i]
                g = fkb.lincomb(ga, fkb.neg(gb))
                sg_t = fkb.fresh()
                nc.scalar.activation(sg_t, g.ap, AF.Sign, bias=zerocol[:], scale=float(g.sg))
                nc.vector.tensor_tensor(pose[:, :, 3 + i], sg_t, mag, OP.mult)

            nc.sync.dma_start(out.rearrange("(p r) c -> p r c", p=P), pose[:])

    nc.compile()
    return nc


_PROG = None


def _get_prog():
    global _PROG
    if _PROG is None:
        _PROG = build_program()
    return _PROG


def kernel(latent_variable, time, W1, b1, W2, b2, W3, b3):
    nc = _get_prog()
    latent_variable = np.ascontiguousarray(latent_variable, dtype=np.float32)
    time = np.ascontiguousarray(time, dtype=np.float32)
    shared = {
        "w1": np.ascontiguousarray(W1, dtype=np.float32),
        "b1": np.ascontiguousarray(b1, dtype=np.float32),
        "w2": np.ascontiguousarray(W2, dtype=np.float32),
        "b2": np.ascontiguousarray(b2, dtype=np.float32),
        "w3": np.ascontiguousarray(W3, dtype=np.float32),
        "b3": np.ascontiguousarray(b3, dtype=np.float32),
    }
    in_maps = []
    for c in range(N_CORES):
        sl = slice(c * B, (c + 1) * B)
        m = {"latent": latent_variable[sl], "time_in": time[sl]}
        m.update(shared)
        in_maps.append(m)
    res = run_bass_kernel_spmd(nc, in_maps, list(range(N_CORES)), **_RUN_KWARGS)
    global LAST_RESULT
    LAST_RESULT = res
    return np.concatenate(
        [res.results[c]["out"] for c in range(N_CORES)], axis=0
    ).astype(np.float32)


LAST_RESULT = None
_RUN_KWARGS = {}
